# revision 1
# baseline (speedup 1.0000x reference)
"""Trainium2 kernel for nn_EulerBias: exact Riemann-solver bias field.

Structure:
  * Host (numpy, float32): the K-interface Newton solve (tiny: B x 63) ->
    wave speeds, then per-batch coefficient matrices for the device stage.
  * Device (8 NeuronCores, batch-parallel, 2 batches/core): for every query
    point q the bias over the 64 segment columns is

        out[q,k] = min(T1[q,k],0) + min(T2[q,k],0)

    where T1/T2 are affine in (u,it,1) = (x*it, 1/(t+eps), 1) with per-k
    coefficients -> one small-contraction (Kc=12) f32 matmul on TensorE
    produces T1||T2 for 512 queries per instruction; ScalarE computes
    relu(-T2); VectorE fuses min(T1,0) - relu(-T2) in one op; DMA stores
    512KB contiguous blocks.

Masked columns (pieces_mask == 0) are encoded in the coefficients
(T1 = -1e9, T2 = +1e30) so no separate mask pass is needed. Assumes
pieces_mask >= 0 (it is a 0/1 mask; the harness fills ones).
"""

import numpy as np

GAMMA = np.float32(1.4)
EPS = np.float32(1e-6)
N_NEWTON = 20
B, K, NT, NX = 16, 64, 128, 256
NQ = NT * NX            # 32768 queries per batch
N_CORES = 8
B_PER_CORE = B // N_CORES
# device tiling
CHUNK = 128             # queries per output-partition group
GROUPS = 4              # chunks per matmul (stationary rows = 3*GROUPS = 12)
SUPER = 4               # matmuls per supertile (PSUM banks)
Q_SUPER = CHUNK * GROUPS * SUPER          # 2048 queries per supertile
ST_PER_BATCH = NQ // Q_SUPER              # 16
BIG = np.float32(1e30)
NEGBIG = np.float32(-1e9)

_COMPILED = None


def _f32(x):
    return np.asarray(x, dtype=np.float32)


def _host_wave_speeds(xs, ks, ks_v, ks_p):
    """Mirror of reference.py's f32 Newton solve, in numpy float32."""
    gm1 = np.float32(GAMMA - 1.0)
    gp1 = np.float32(GAMMA + 1.0)
    exp_rare = np.float32(gm1 / (2.0 * GAMMA))

    def clip_lo(v, lo=EPS):
        return np.maximum(v, lo)

    rho_L, rho_R = ks[:, :-1], ks[:, 1:]
    u_L, u_R = ks_v[:, :-1], ks_v[:, 1:]
    p_L, p_R = ks_p[:, :-1], ks_p[:, 1:]

    def sound(rho, p):
        return np.sqrt(clip_lo(GAMMA * p / clip_lo(rho)))

    c_L, c_R = sound(rho_L, p_L), sound(rho_R, p_R)
    A_L = np.float32(2.0) / (gp1 * clip_lo(rho_L))
    A_R = np.float32(2.0) / (gp1 * clip_lo(rho_R))
    B_L = gm1 / gp1 * p_L
    B_R = gm1 / gp1 * p_R

    def wave_f_df(p, p_K, A_K, B_K, c_K):
        denom = clip_lo(p + B_K)
        sqrt_AoD = np.sqrt(clip_lo(A_K / denom))
        f_shock = (p - p_K) * sqrt_AoD
        df_shock = sqrt_AoD * (np.float32(1.0) - (p - p_K) / (np.float32(2.0) * denom))
        p_ratio = clip_lo(p / clip_lo(p_K))
        f_rare = np.float32(2.0) * c_K / gm1 * (p_ratio ** exp_rare - np.float32(1.0))
        df_rare = c_K / (GAMMA * clip_lo(p_K)) * p_ratio ** np.float32(-gp1 / (2.0 * GAMMA))
        is_shock = p > p_K
        return np.where(is_shock, f_shock, f_rare), np.where(is_shock, df_shock, df_rare)

    p0 = clip_lo(((c_L + c_R - gm1 / np.float32(2.0) * (u_R - u_L))
                  / (c_L / clip_lo(p_L) ** exp_rare + c_R / clip_lo(p_R) ** exp_rare))
                 ** np.float32(1.0 / exp_rare))
    p_star = p0
    for _ in range(N_NEWTON):
        f_L, df_L = wave_f_df(p_star, p_L, A_L, B_L, c_L)
        f_R, df_R = wave_f_df(p_star, p_R, A_R, B_R, c_R)
        residual = f_L + f_R + (u_R - u_L)
        jacobian = clip_lo(df_L + df_R)
        p_star = clip_lo(p_star - residual / jacobian)

    gp1_o_2g = np.float32(gp1 / (2.0 * GAMMA))
    sigma_1 = u_L - c_L * np.sqrt(clip_lo(np.float32(1.0) + gp1_o_2g * (p_star / clip_lo(p_L) - np.float32(1.0))))
    speed_left = np.where(p_star > p_L, sigma_1, u_L - c_L)
    sigma_3 = u_R + c_R * np.sqrt(clip_lo(np.float32(1.0) + gp1_o_2g * (p_star / clip_lo(p_R) - np.float32(1.0))))
    speed_right = np.where(p_star > p_R, sigma_3, u_R + c_R)
    return speed_left.astype(np.float32), speed_right.astype(np.float32)


def _host_coef(xs, mask, sl, sr):
    """Per-batch [12, 512] moving-operand coefficient matrices.

    psum col n = 64*j + k      (j = chunk-in-group) -> T1 = -m*relu-arg form
    psum col n = 256 + 64*j + k                     -> T2
    contraction rows 3j+(0,1,2) multiply (u, it, 1) of chunk j.
    """
    xd = xs[:, 1:K]                      # (B, 63)
    m = mask.astype(np.float32)          # (B, 64)
    act = m != 0

    # T1 = -m*u + m*xd*it + m*sr   (k < 63);  col 63 -> +BIG;  masked -> -1e9 const
    Wu1 = np.zeros((B, K), np.float32)
    Wi1 = np.zeros((B, K), np.float32)
    Wc1 = np.zeros((B, K), np.float32)
    Wu1[:, :63] = -m[:, :63]
    Wi1[:, :63] = m[:, :63] * xd
    Wc1[:, :63] = m[:, :63] * sr
    Wc1[:, 63] = BIG
    Wu1[~act] = 0.0
    Wi1[~act] = 0.0
    Wc1[~act] = NEGBIG

    # T2 = m*u - m*xd[k-1]*it - m*sl[k-1] (k >= 1); col 0 or masked -> +BIG
    # (so min(T2,0) = -m*relu(sl[k-1] - xi[k-1]))
    Wu2 = np.zeros((B, K), np.float32)
    Wi2 = np.zeros((B, K), np.float32)
    Wc2 = np.zeros((B, K), np.float32)
    Wu2[:, 1:] = m[:, 1:]
    Wi2[:, 1:] = -m[:, 1:] * xd
    Wc2[:, 1:] = -m[:, 1:] * sl
    Wc2[:, 0] = BIG
    Wu2[~act] = 0.0
    Wi2[~act] = 0.0
    Wc2[~act] = BIG

    coef = np.zeros((B, 3 * GROUPS, 512), np.float32)
    for j in range(GROUPS):
        c1 = slice(64 * j, 64 * j + 64)
        c2 = slice(256 + 64 * j, 256 + 64 * j + 64)
        coef[:, 3 * j + 0, c1] = Wu1
        coef[:, 3 * j + 1, c1] = Wi1
        coef[:, 3 * j + 2, c1] = Wc1
        coef[:, 3 * j + 0, c2] = Wu2
        coef[:, 3 * j + 1, c2] = Wi2
        coef[:, 3 * j + 2, c2] = Wc2
    return coef


def _host_qdata(t_coords, x_coords):
    """(B, ST/2, 12, 1024) stationary operands: rows 3j+(0,1,2) = (u, it, 1),
    column 512*h + 128*g + m.

    Query assignment q(sp, h, g, j, m) = sp*4096 + h*2048 + m*16 + g*4 + j, so
    each half-supertile's partition-major store walk (m, (g j), k) writes one
    monotonically contiguous 512KB HBM range (4KB per partition row).
    512KB store granularity measured faster than 1MB/2MB (smoother overlap);
    one 48KB qd load per supertile-pair halves load instructions on the
    store ring."""
    it = np.float32(1.0) / (t_coords.reshape(B, NQ) + EPS)
    u = x_coords.reshape(B, NQ) * it

    def lay(v):
        # (b, sp, h, m, g, j) -> [b, sp, j, (h, g, m)]
        v = v.reshape(B, ST_PER_BATCH // 2, 2, CHUNK, SUPER, GROUPS)
        return np.transpose(v, (0, 1, 5, 2, 4, 3)).reshape(
            B, ST_PER_BATCH // 2, GROUPS, 2 * SUPER * CHUNK)

    qd = np.empty((B, ST_PER_BATCH // 2, 3 * GROUPS, 2 * SUPER * CHUNK), np.float32)
    qd[:, :, 0::3, :] = lay(u)
    qd[:, :, 1::3, :] = lay(it)
    qd[:, :, 2::3, :] = 1.0
    return qd


def _build_nc():
    import concourse.bacc as bacc
    import concourse.mybir as mybir
    import concourse.tile as tile

    nc = bacc.Bacc(None, target_bir_lowering=False, debug=False)
    qd_d = nc.declare_dram_parameter(
        "qd", [B_PER_CORE, ST_PER_BATCH // 2, 3 * GROUPS, 2 * SUPER * CHUNK],
        mybir.dt.float32, isOutput=False)
    cf_d = nc.declare_dram_parameter(
        "cf", [B_PER_CORE, 3 * GROUPS, 512], mybir.dt.float32, isOutput=False)
    out_d = nc.declare_dram_parameter(
        "out", [B_PER_CORE, NQ, K], mybir.dt.float32, isOutput=True)

    f32 = mybir.dt.float32
    with tile.TileContext(nc) as tc:
        with (
            tc.tile_pool(name="cf", bufs=1) as cfp,
            tc.tile_pool(name="qd", bufs=4) as qdp,
            tc.tile_pool(name="ps", bufs=4, space="PSUM") as psp,
            tc.tile_pool(name="p2", bufs=6) as p2p,
            tc.tile_pool(name="ot", bufs=8) as otp,
        ):
            cft = []
            for b in range(B_PER_CORE):
                c = cfp.tile([3 * GROUPS, 512], f32, tag=f"cf{b}")
                nc.sync.dma_start(c[:], cf_d[b])
                cft.append(c)
            for b in range(B_PER_CORE):
                for sp in range(ST_PER_BATCH // 2):
                    qdt = qdp.tile([3 * GROUPS, 2 * SUPER * CHUNK], f32)
                    nc.sync.dma_start(qdt[:], qd_d[b, sp])
                    for h in range(2):
                        ot = otp.tile([128, SUPER, 256], f32)
                        # two 2-bank PSUM tiles per half: deeper PE pipelining
                        # and finer ACT/STT quanta that overlap the stores.
                        for half in range(2):
                            ps = psp.tile([128, 2, 512], f32)
                            for g2 in range(2):
                                g = half * 2 + g2
                                nc.tensor.matmul(
                                    ps[:, g2, :],
                                    qdt[:, 512 * h + 128 * g:512 * h + 128 * (g + 1)],
                                    cft[b][:],
                                    start=True, stop=True,
                                )
                            p2 = p2p.tile([128, 2, 256], f32)
                            nc.scalar.activation(
                                p2[:], ps[:, :, 256:512],
                                mybir.ActivationFunctionType.Relu, scale=-1.0)
                            nc.vector.scalar_tensor_tensor(
                                out=ot[:, half * 2:half * 2 + 2],
                                in0=ps[:, :, 0:256], scalar=0.0, in1=p2[:],
                                op0=mybir.AluOpType.min,
                                op1=mybir.AluOpType.subtract)
                        q0 = sp * 2 * Q_SUPER + h * Q_SUPER
                        dst = out_d[b, q0:q0 + Q_SUPER, :].rearrange(
                            "(m c) k -> m c k", c=SUPER * GROUPS)
                        src = ot[:].rearrange("m g (j k) -> m (g j) k", k=K)
                        nc.sync.dma_start(dst, src)
    nc.compile()
    return nc


def _get_compiled():
    global _COMPILED
    if _COMPILED is None:
        _COMPILED = _build_nc()
    return _COMPILED


def run(inputs, trace=False):
    from concourse.bass_utils import run_bass_kernel_spmd

    xs = _f32(inputs["xs"])
    ks = _f32(inputs["ks"])
    ks_v = _f32(inputs["ks_v"])
    ks_p = _f32(inputs["ks_p"])
    mask = _f32(inputs["pieces_mask"])
    t_coords = _f32(inputs["t_coords"])
    x_coords = _f32(inputs["x_coords"])

    sl, sr = _host_wave_speeds(xs, ks, ks_v, ks_p)
    coef = _host_coef(xs, mask, sl, sr)
    qd = _host_qdata(t_coords, x_coords)

    nc = _get_compiled()
    in_maps = [
        {
            "qd": np.ascontiguousarray(qd[c * B_PER_CORE:(c + 1) * B_PER_CORE]),
            "cf": np.ascontiguousarray(coef[c * B_PER_CORE:(c + 1) * B_PER_CORE]),
        }
        for c in range(N_CORES)
    ]
    res = None
    for attempt in range(3):
        try:
            res = run_bass_kernel_spmd(
                nc, in_maps, core_ids=list(range(N_CORES)), trace=trace)
            break
        except Exception:
            if attempt == 2:
                raise
            import time as _time
            _time.sleep(2.0)
    out = np.empty((B, NT, NX, K), np.float32)
    for c in range(N_CORES):
        out[c * B_PER_CORE:(c + 1) * B_PER_CORE] = (
            res.results[c]["out"].reshape(B_PER_CORE, NT, NX, K))
    return out, res


def kernel(**inputs):
    out, _ = run(inputs, trace=False)
    return out

